# revision 1
# baseline (speedup 1.0000x reference)
"""Causal self-attention (T=2048, C=1024, H=16) on 8 trn2 NeuronCores.

Tensor-parallel over heads: core i computes heads 2i, 2i+1 (q/k/v rows
128i:128i+128 of each 1024-row block of wqkv_w, proj_w columns
128i:128i+128), producing a partial output projection; partials are summed
on the host (the all-reduce of the sharding hint).

Per-core Bass/Tile kernel, bf16 matmuls with fp32 PSUM accumulation. The
PE instruction stream is kept dense (HAM re-throttles the PE clock to
1.2GHz after any ~3.4us idle/transpose-only window, and only re-warms
after ~3.4us of continuous matmul activity):
  B. qkvT[j, t] = wqkv.T @ xT, contraction-tile outer so matmuls chase the
     x DMAs; evacuation adds the (per-partition) bias and casts to bf16;
     q rows pre-scaled by 1/sqrt(C) on the host. v's PE transposes into
     v_aug[k, 65] (ones column = softmax denominator via the PV matmul)
     are interleaved with the v matmuls so they never form a
     transpose-only PE window.
  D. per 512-col t-chunk, heads interleaved, PV pipelined PIPE k-tile
     steps behind the scores, and the previous chunk's normalize/proj
     matmuls spread through the j-loop as PE filler while ScalarE
     (exp) catches up:
       sT[k, t] = kT.T @ qT -> exp (bf16, no max-subtraction needed:
       |scores| < ~1 by construction) -> causal affine_select on gpsimd
       (diagonal k-tiles only; fully-masked tiles skipped) ->
       attnT[d, t] += v_aug.T @ exp_w.
     Normalize: 1/sums via Ln+Exp(-x) on ScalarE (one activation table
     set holds both), partition-broadcast via K=1 float32r matmul with
     ones, one DVE multiply.
  E. partialT[o, t] = projT.T @ attn (two K=64 matmuls, one per head).
"""

import sys

if "/opt/trn_rl_repo" not in sys.path:
    sys.path.insert(0, "/opt/trn_rl_repo")

import ml_dtypes
import numpy as np

T = 2048
C = 1024
CH = 512  # t-chunk width (one PSUM bank of fp32)
NT = T // CH  # 4 t-chunks
NK = T // 128  # 16 k-tiles
NCT = C // 128  # 8 contraction tiles
N_CORES = 8
PIPE = 4  # scores->PV pipeline depth in k-tile steps

_CACHE = {}


def _patch_act_tables(bacc_mod, mybir):
    """Make Exp and Ln resolve to the one table set containing both, so the
    kernel needs a single ACT_TABLE_LOAD instead of thrashing between
    exp_and_others and natural_log_exp_and_others (~1.3us per reload)."""
    if getattr(bacc_mod, "_attn_act_patch", False):
        return
    orig = bacc_mod.get_activation_tables
    both = {mybir.ActivationFunctionType.Exp, mybir.ActivationFunctionType.Ln}

    def patched(arch):
        tabs = dict(orig(arch))
        return {
            name: (funcs if name == "natural_log_exp_and_others" else funcs - both)
            for name, funcs in tabs.items()
        }

    bacc_mod.get_activation_tables = patched
    bacc_mod._attn_act_patch = True


def _build():
    import concourse.tile as tile
    from concourse import bacc, mybir

    _patch_act_tables(bacc, mybir)

    F32 = mybir.dt.float32
    F32R = mybir.dt.float32r
    BF16 = mybir.dt.bfloat16
    EXP = mybir.ActivationFunctionType.Exp
    LN = mybir.ActivationFunctionType.Ln
    IS_GE = mybir.AluOpType.is_ge

    nc = bacc.Bacc(
        "TRN2",
        target_bir_lowering=False,
        debug=False,
        enable_asserts=False,
        num_devices=N_CORES,
        num_swdge_queues=4,
    )
    xT = nc.dram_tensor("xT", [C, T], BF16, kind="ExternalInput").ap()
    wqkv = nc.dram_tensor("wqkv", [C, 384], BF16, kind="ExternalInput").ap()
    projT = nc.dram_tensor("projT", [128, C], BF16, kind="ExternalInput").ap()
    identb = nc.dram_tensor("identb", [128, 64], BF16, kind="ExternalInput").ap()
    ones_f = nc.dram_tensor("ones_f", [128, 128], F32R, kind="ExternalInput").ap()
    bias = nc.dram_tensor("bias", [128, 3], F32, kind="ExternalInput").ap()
    # output as contiguous [chunk, o-tile, 128, 512] bf16 tiles: each store is
    # one fully-contiguous 128KB DMA (strided stores run at ~1/3 the queue
    # rate, and bf16 halves both the store bytes and the evacuation time)
    out = nc.dram_tensor("out", [NT, 8, 128, CH], BF16, kind="ExternalOutput").ap()

    with tile.TileContext(nc) as tc:
        with (
            tc.tile_pool(name="big", bufs=1) as big,
            tc.tile_pool(name="expw", bufs=12) as expw_pool,
            tc.tile_pool(name="attn_tmp", bufs=2) as attn_tmp_pool,
            tc.tile_pool(name="outev", bufs=3) as outev_pool,
            tc.tile_pool(name="ps", bufs=1, space="PSUM") as ps,
        ):
            # ---- resident SBUF tensors -------------------------------------
            x_sb = big.tile([128, NCT, T], BF16, name="x_sb")
            w_sb = big.tile([128, NCT, 384], BF16, name="w_sb")
            proj0_sb = big.tile([64, C], BF16, name="proj0_sb")
            proj1_sb = big.tile([64, C], BF16, name="proj1_sb")
            qT_sb = big.tile([128, T], BF16, name="qT_sb")
            kT_sb = big.tile([128, T], BF16, name="kT_sb")
            vT_sb = big.tile([128, T], BF16, name="vT_sb")
            v_aug0 = big.tile([128, NK, 65], BF16, name="v_aug0")
            v_aug1 = big.tile([128, NK, 65], BF16, name="v_aug1")
            attn0 = big.tile([64, T], BF16, name="attn0")
            attn1 = big.tile([64, T], BF16, name="attn1")
            ident_sb = big.tile([128, 64], BF16, name="ident_sb")
            ones_sb = big.tile([128, 128], F32R, name="ones_sb")
            bias_sb = big.tile([128, 3], F32, name="bias_sb")

            # x and w tiles are fully-contiguous DRAM regions (full rows), so
            # each DMA streams at queue peak; pairs ordered so stage B's
            # matmuls start as soon as the first pair lands.
            # x tiles alternate between the two HWDGE queues (sync + scalar —
            # ScalarE is idle this early) so the load streams at 2x one
            # queue's ~100GB/s.
            for ct in range(NCT):
                nc.sync.dma_start(
                    out=w_sb[:, ct, :], in_=wqkv[128 * ct : 128 * ct + 128, :]
                )
                # scalar's queue measured ~2x sync's rate: give it the six
                # tiles stage B consumes first, sync the last two
                xeng = nc.sync if ct >= 6 else nc.scalar
                xeng.dma_start(
                    out=x_sb[:, ct, :], in_=xT[128 * ct : 128 * ct + 128, :]
                )
            nc.sync.dma_start(out=bias_sb, in_=bias)
            nc.sync.dma_start(out=ident_sb, in_=identb)
            nc.sync.dma_start(out=ones_sb, in_=ones_f)
            nc.sync.dma_start(out=proj0_sb, in_=projT[0:64, :])
            nc.sync.dma_start(out=proj1_sb, in_=projT[64:128, :])

            nc.vector.memset(v_aug0[:, :, 64:65], 1.0)
            nc.vector.memset(v_aug1[:, :, 64:65], 1.0)

            # ---- stage B: q/k projections, two ct-outer sweeps -------------
            for sweep in range(2):
                grp = {}
                slots = [("s", 4), ("s", 4), ("s", 4), ("s", 4)]
                for part in (0, 1):
                    for c in (2 * sweep, 2 * sweep + 1):
                        tag, nbufs = slots.pop(0)
                        grp[(part, c)] = ps.tile(
                            [128, CH],
                            F32,
                            tag=tag,
                            bufs=nbufs,
                            name=f"qkps_{part}_{c}",
                        )
                for ct in range(NCT):
                    for (part, c), g in grp.items():
                        cols = slice(128 * part, 128 * part + 128)
                        nc.tensor.matmul(
                            g,
                            w_sb[:, ct, cols],
                            x_sb[:, ct, CH * c : CH * c + CH],
                            start=(ct == 0),
                            stop=(ct == NCT - 1),
                        )
                for (part, c), g in grp.items():
                    dest = qT_sb if part == 0 else kT_sb
                    nc.vector.tensor_scalar_add(
                        dest[:, CH * c : CH * c + CH], g, bias_sb[:, part : part + 1]
                    )

            # v projection + PE transposes, per chunk; chunk 0 up front, the
            # rest emitted as PE filler inside stage D's j-loops.
            def emit_v_chunk(c):
                v_ps = ps.tile([128, CH], F32, tag="m", bufs=2, name=f"vps_{c}")
                for ct in range(NCT):
                    nc.tensor.matmul(
                        v_ps,
                        w_sb[:, ct, 256:384],
                        x_sb[:, ct, CH * c : CH * c + CH],
                        start=(ct == 0),
                        stop=(ct == NCT - 1),
                    )
                nc.vector.tensor_scalar_add(
                    vT_sb[:, CH * c : CH * c + CH], v_ps, bias_sb[:, 2:3]
                )

            def transposes_for(c):
                for h, v_aug in ((0, v_aug0), (1, v_aug1)):
                    hrow = slice(64 * h, 64 * h + 64)
                    for kt in range(4 * c, 4 * c + 4):
                        tr_ps = ps.tile(
                            [128, 64], BF16, tag="m", bufs=2, name=f"tr_{h}_{kt}"
                        )
                        nc.tensor.transpose(
                            tr_ps,
                            vT_sb[hrow, 128 * kt : 128 * kt + 128],
                            ident_sb[hrow, :],
                        )
                        nc.vector.tensor_copy(v_aug[:, kt, 0:64], tr_ps)

            emit_v_chunk(0)
            transposes_for(0)
            emit_v_chunk(1)
            transposes_for(1)

            # ---- stages D+E per t-chunk ------------------------------------
            # Deferred work from chunk c-1, spread through chunk c's j-loop.
            pending_norm = None  # (at2, chunk)
            pending_proj = None  # chunk index

            def emit_norm(at2, pc):
                tcol = slice(CH * pc, CH * pc + CH)
                for h, attn in ((0, attn0), (1, attn1)):
                    rb_ps = ps.tile([128, CH], F32, tag="m", bufs=2, name=f"rb_{h}_{pc}")
                    nc.tensor.matmul(
                        rb_ps,
                        ones_sb[64:65, :],
                        at2[64:65, CH * h : CH * h + CH],
                        start=True,
                        stop=True,
                    )
                    nc.vector.tensor_mul(
                        attn[:, tcol], at2[0:64, CH * h : CH * h + CH], rb_ps[0:64, :]
                    )

            def emit_proj_tile(pc, m):
                tcol = slice(CH * pc, CH * pc + CH)
                pr_ps = ps.tile([128, CH], F32, tag="m", bufs=2, name=f"pr_{m}_{pc}")
                nc.tensor.matmul(
                    pr_ps,
                    proj0_sb[:, 128 * m : 128 * m + 128],
                    attn0[:, tcol],
                    start=True,
                    stop=False,
                )
                nc.tensor.matmul(
                    pr_ps,
                    proj1_sb[:, 128 * m : 128 * m + 128],
                    attn1[:, tcol],
                    start=False,
                    stop=True,
                )
                ob = outev_pool.tile([128, CH], BF16, tag="outev", name=f"ob_{m}_{pc}")
                nc.vector.tensor_copy(ob, pr_ps)
                nc.sync.dma_start(out=out[pc, m], in_=ob)

            # Chunk order (1, 2, 3, 0): the ScalarE-heavy late chunks get the
            # previous chunk's projection matmuls as PE filler, and the final
            # chunk processed (0) has the shortest tail.
            for c in (1, 2, 3, 0):
                tcol = slice(CH * c, CH * c + CH)
                nj = 4 * c + 4
                pv_ps = {
                    h: ps.tile([65, CH], F32, tag="pv", bufs=2, name=f"pv_{h}_{c}")
                    for h in (0, 1)
                }
                pending = []
                proj_emitted = 0

                def emit_pv(step, last):
                    for (pj, ph, pw, plo) in step:
                        nc.tensor.matmul(
                            pv_ps[ph][:, plo:CH],
                            (v_aug0 if ph == 0 else v_aug1)[:, pj, :],
                            pw[:, plo:CH],
                            start=(pj == 0),
                            stop=last,
                        )

                for j in range(nj):
                    for h in (0, 1):
                        hrow = slice(64 * h, 64 * h + 64)
                        s_ps = ps.tile(
                            [128, CH], F32, tag="s", bufs=4, name=f"s_{h}_{c}_{j}"
                        )
                        # diagonal tiles: columns < 128*diag are fully masked
                        # downstream, so don't compute their scores either
                        slo = max(0, 128 * (j - 4 * c))
                        nc.tensor.matmul(
                            s_ps[:, slo:CH],
                            kT_sb[hrow, 128 * j : 128 * j + 128],
                            qT_sb[hrow, CH * c + slo : CH * c + CH],
                            start=True,
                            stop=True,
                        )
                        w_t = expw_pool.tile(
                            [128, CH], BF16, tag="expw", name=f"w_{h}_{c}_{j}"
                        )
                        diag = j - 4 * c
                        if diag <= 0:
                            nc.scalar.activation(out=w_t, in_=s_ps, func=EXP)
                        else:
                            # columns < 128*diag are fully masked: skip their
                            # exp; PV also skips them (zero contribution)
                            lo = 128 * diag
                            nc.scalar.activation(
                                out=w_t[:, lo:CH], in_=s_ps[:, lo:CH], func=EXP
                            )
                        if diag >= 0:
                            # keep exp(score) where t >= k: within the kept
                            # column range f' = f - 128*diag, so f' - p >= 0
                            lo = 128 * diag if diag > 0 else 0
                            nc.gpsimd.affine_select(
                                out=w_t[:, lo:CH],
                                in_=w_t[:, lo:CH],
                                pattern=[[1, CH - lo]],
                                compare_op=IS_GE,
                                fill=0.0,
                                base=0,
                                channel_multiplier=-1,
                            )
                        pending.append((j, h, w_t, max(0, 128 * diag)))
                    if j == 0 and pending_norm is not None:
                        emit_norm(*pending_norm)
                        pending_norm = None
                    if j == 1 and c == 1:
                        emit_v_chunk(2)
                    if j == 2 and c == 1:
                        transposes_for(2)
                    if j == 1 and c == 3:
                        emit_v_chunk(3)
                    if j == 3 and c == 3:
                        transposes_for(3)
                    while len(pending) > 2 * PIPE:
                        step, pending = pending[:2], pending[2:]
                        emit_pv(step, last=False)
                    if pending_proj is not None and j >= 1:
                        target = (j * 8) // max(nj - 1, 1)
                        while proj_emitted < min(target, 8):
                            emit_proj_tile(pending_proj, proj_emitted)
                            proj_emitted += 1
                while pending:
                    step, pending = pending[:2], pending[2:]
                    emit_pv(step, last=(len(pending) == 0))
                if pending_proj is not None:
                    while proj_emitted < 8:
                        emit_proj_tile(pending_proj, proj_emitted)
                        proj_emitted += 1

                at2 = attn_tmp_pool.tile(
                    [65, 2 * CH], F32R, tag="attn_tmp", name=f"at2_{c}"
                )
                for h in (0, 1):
                    nc.vector.tensor_copy(at2[:, CH * h : CH * h + CH], pv_ps[h])
                # 1/sums = exp(-ln(sums)); Ln+Exp share one act table set
                nc.scalar.activation(out=at2[64:65, :], in_=at2[64:65, :], func=LN)
                nc.scalar.activation(
                    out=at2[64:65, :], in_=at2[64:65, :], func=EXP, scale=-1.0
                )
                pending_norm = (at2, c)
                pending_proj = c

            emit_norm(*pending_norm)
            for m in range(8):
                emit_proj_tile(pending_proj, m)

    nc.compile()
    return nc


def _get_nc():
    if "nc" not in _CACHE:
        _CACHE["nc"] = _build()
    return _CACHE["nc"]


def _make_in_maps(x, wqkv_w, wqkv_b, proj_w):
    bf = ml_dtypes.bfloat16
    xT = np.ascontiguousarray(np.asarray(x, np.float32).T.astype(bf))
    identb = np.zeros((128, 64), bf)
    identb[0:64, :] = np.eye(64, dtype=bf)
    identb[64:128, :] = np.eye(64, dtype=bf)
    ones_f = np.ones((128, 128), np.float32)
    scale = np.float32(1.0 / np.sqrt(C))
    in_maps = []
    for i in range(N_CORES):
        rows = []
        biases = []
        for blk, s in ((0, scale), (1, None), (2, None)):
            sl = slice(blk * C + 128 * i, blk * C + 128 * i + 128)
            w = np.asarray(wqkv_w[sl], np.float32)
            b = np.asarray(wqkv_b[sl], np.float32)
            if s is not None:
                w = w * s
                b = b * s
            rows.append(w)
            biases.append(b)
        W = np.concatenate(rows, axis=0)  # [384, 1024]
        B = np.stack(biases, axis=1)  # [128, 3]
        pT = np.asarray(proj_w[:, 128 * i : 128 * i + 128], np.float32).T  # [128, 1024]
        in_maps.append(
            {
                "xT": xT,
                "wqkv": np.ascontiguousarray(W.T.astype(bf)),
                "projT": np.ascontiguousarray(pT.astype(bf)),
                "identb": identb,
                "ones_f": ones_f,
                "bias": np.ascontiguousarray(B),
            }
        )
    return in_maps


def kernel(x, wqkv_w, wqkv_b, proj_w, proj_b, _trace=False, _tmpdir=None):
    from concourse.bass_utils import run_bass_kernel_spmd

    nc = _get_nc()
    in_maps = _make_in_maps(x, wqkv_w, wqkv_b, proj_w)
    res = run_bass_kernel_spmd(
        nc,
        in_maps,
        core_ids=list(range(N_CORES)),
        trace=_trace,
        tmpdir=_tmpdir,
    )
    acc = np.zeros((NT, 8, 128, CH), np.float64)
    for rmap in res.results:
        acc += rmap["out"].astype(np.float64)
    partialT = acc.transpose(1, 2, 0, 3).reshape(C, T)  # [o, t]
    full = partialT.T + np.asarray(proj_b, np.float64)[None, :]
    if _trace:
        _CACHE["last_result"] = res
    return full.astype(np.float32)



# revision 40
# speedup vs baseline: 1.0056x; 1.0056x over previous
"""Causal self-attention (T=2048, C=1024, H=16) on 8 trn2 NeuronCores.

Tensor-parallel over heads: core i computes heads 2i, 2i+1 (q/k/v rows
128i:128i+128 of each 1024-row block of wqkv_w, proj_w columns
128i:128i+128), producing a partial output projection; partials are summed
on the host (the all-reduce of the sharding hint).

Per-core Bass/Tile kernel, bf16 matmuls with fp32 PSUM accumulation.
Layout puts head 0's attention pipeline on partitions 0-63 and head 1's on
64-127 so the two heads' K=64 matmuls land in disjoint PE row groups
(auto tile_position from base partitions) and run concurrently:
  B. warmup matmuls on the identity tile keep HAM's activity window busy
     while the x DMAs stream, so stage B starts at K=8/8 (2.4GHz).
     qkvT[j, t] = wqkv.T @ xT, contraction-tile outer so matmuls chase the
     x DMAs; q rows pre-scaled by 1/sqrt(C) on the host. v's 128x128 PE
     transposes produce both heads' v_aug tiles at once and are interleaved
     with the v matmuls so they never form a transpose-only PE window.
  D. per 512-col t-chunk: sT[k, 2, t] = kT.T @ qT (both heads, one wide
     2-bank PSUM tile) -> ONE wide exp per j on ScalarE (bf16 out, no
     max-subtraction: |scores| < ~1) -> causal affine_select on gpsimd
     (diagonal k-tiles only, both heads in one 3D op) -> PV:
       pvA[0:65]  += v_aug0.T @ w0   (M=65, ones col = head-0 denominator)
       den1[96:97]+= ones.T   @ w1   (M=1 packed into PE col group 3,
                                      concurrent with the pvA matmul)
       pvB[64:128]+= v_aug1.T @ w1   (M=64 at base partition 64)
     Normalize on DVE: denominators broadcast via two concurrent K=1
     matmuls (row groups 2/3), reciprocal_approx_fast, one multiply per
     head -> attn_pair[128, T] bf16 (h0 rows 0-63, h1 rows 64-127).
     Previous chunk's normalize/proj spread through the j-loop as PE
     filler while ScalarE (exp) is the per-step bottleneck.
  E. partialT[o, t] = projT.T @ attn_pair: per o-tile, two concurrent
     K=64 matmuls (row groups {2,3} then {0,1}; the rows-64-127 one is
     emitted first / start=True because its drain path is shorter).
"""

import sys

if "/opt/trn_rl_repo" not in sys.path:
    sys.path.insert(0, "/opt/trn_rl_repo")

import os

import ml_dtypes
import numpy as np

_DBG = set(os.environ.get("KDBG", "").split(","))


class _FakeWide:
    """Pair of 1-bank PSUM tiles indexed like a [128, 2, CH] wide tile."""

    def __init__(self, tiles):
        self.tiles = tiles

    def __getitem__(self, idx):
        assert len(idx) == 3
        return self.tiles[idx[1]][idx[0], idx[2]]

T = 2048
C = 1024
CH = 512  # t-chunk width (one PSUM bank of fp32)
NT = T // CH  # 4 t-chunks
NK = T // 128  # 16 k-tiles
NCT = C // 128  # 8 contraction tiles
N_CORES = 8
PIPE = 3  # scores->PV pipeline depth in j-steps
N_WARM = 34  # warmup matmuls (N=128 cold ~107ns each => ~3.6us, one HAM window)

_CACHE = {}


def _build():
    import concourse.tile as tile
    from concourse import bacc, mybir

    F32 = mybir.dt.float32
    F32R = mybir.dt.float32r
    BF16 = mybir.dt.bfloat16
    EXP = mybir.ActivationFunctionType.Exp
    IS_GE = mybir.AluOpType.is_ge

    nc = bacc.Bacc(
        "TRN2",
        target_bir_lowering=False,
        debug=False,
        enable_asserts=False,
        num_devices=N_CORES,
        num_swdge_queues=4,
    )
    xT = nc.dram_tensor("xT", [C, T], BF16, kind="ExternalInput").ap()
    wqkv = nc.dram_tensor("wqkv", [C, 384], BF16, kind="ExternalInput").ap()
    projT = nc.dram_tensor("projT", [128, C], BF16, kind="ExternalInput").ap()
    identb = nc.dram_tensor("identb", [128, 128], BF16, kind="ExternalInput").ap()
    normones = nc.dram_tensor("normones", [128, 128], F32R, kind="ExternalInput").ap()
    bias = nc.dram_tensor("bias", [128, 3], F32, kind="ExternalInput").ap()
    # output as contiguous [chunk, o-tile, 128, 512] bf16 tiles: each store is
    # one fully-contiguous 128KB DMA
    out = nc.dram_tensor("out", [NT, 8, 128, CH], BF16, kind="ExternalOutput").ap()

    with tile.TileContext(nc) as tc:
        with (
            tc.tile_pool(name="big", bufs=1) as big,
            tc.tile_pool(name="expw", bufs=8) as expw_pool,
            tc.tile_pool(name="outev", bufs=3) as outev_pool,
            tc.tile_pool(name="ps", bufs=1, space="PSUM") as ps,
        ):
            # ---- resident SBUF tensors -------------------------------------
            x_sb = big.tile([128, NCT, T], BF16, name="x_sb")
            w_sb = big.tile([128, NCT, 384], BF16, name="w_sb")
            proj0_sb = big.tile([64, C], BF16, name="proj0_sb")
            proj1_sb = big.tile([64, C], BF16, name="proj1_sb")
            qT_sb = big.tile([128, T], BF16, name="qT_sb")
            kT_sb = big.tile([128, T], BF16, name="kT_sb")
            vT_sb = big.tile([128, T], BF16, name="vT_sb")
            v_aug0 = big.tile([128, NK, 65], BF16, name="v_aug0")
            v_aug1 = big.tile([128, NK, 64], BF16, name="v_aug1")
            attn0 = big.tile([64, T], BF16, name="attn0")
            attn1 = big.tile([64, T], BF16, name="attn1")
            ident_sb = big.tile([128, 128], BF16, name="ident_sb")
            onesc_sb = big.tile([128, 1], BF16, name="onesc_sb")
            ones_nrm = big.tile([128, 128], F32R, name="ones_nrm")
            sums_sb = big.tile([128, NT, 2, CH], F32R, name="sums_sb")
            inv_sb = big.tile([64, 2, CH], F32, name="inv_sb")
            bias_sb = big.tile([128, 3], F32, name="bias_sb")
            scr_sb = big.tile([1, 2], F32, name="scr_sb")

            # ident first: the warmup matmuls depend only on it.
            nc.sync.dma_start(out=ident_sb, in_=identb)
            nc.sync.dma_start(out=bias_sb, in_=bias)

            # warmup: keep the PE array busy through one HAM activity window
            # while the big x DMAs stream, so stage B starts at 2.4GHz.
            if "nowarm" not in _DBG:
                warm_ps = ps.tile([128, CH], F32, tag="m", bufs=2, name="warm_ps")
                for _ in range(N_WARM):
                    nc.tensor.matmul(
                        warm_ps[:, 0:128], ident_sb, ident_sb, start=True, stop=True
                    )
            # preload the exp activation table during the DMA ramp
            nc.scalar.activation(out=scr_sb[0:1, 0:1], in_=bias_sb[0:1, 0:1], func=EXP)

            # x and w tiles are fully-contiguous DRAM regions (full rows), so
            # each DMA streams at queue peak; pairs ordered so stage B's
            # matmuls start as soon as the first pair lands.
            for ct in range(NCT):
                nc.sync.dma_start(
                    out=w_sb[:, ct, :], in_=wqkv[128 * ct : 128 * ct + 128, :]
                )
                # scalar's queue measured ~2x sync's rate: give it the six
                # tiles stage B consumes first, sync the last two
                xeng = nc.sync if ct >= 6 else nc.scalar
                xeng.dma_start(
                    out=x_sb[:, ct, :], in_=xT[128 * ct : 128 * ct + 128, :]
                )
            nc.sync.dma_start(out=proj0_sb, in_=projT[0:64, :])
            nc.sync.dma_start(out=proj1_sb, in_=projT[64:128, :])

            nc.sync.dma_start(out=ones_nrm, in_=normones)
            nc.vector.memset(v_aug0[:, :, 64:65], 1.0)
            nc.vector.memset(onesc_sb, 1.0)

            # ---- stage B: q/k projections, two ct-outer sweeps -------------
            # 4 accumulation groups per sweep in 2 wide (2-bank) PSUM slots:
            # slot part 0 = q chunks, part 1 = k chunks.
            for sweep in range(2):
                grp = {}
                for part in (0, 1):
                    if "nowides" in _DBG:
                        for ci in (0, 1):
                            g = ps.tile(
                                [128, CH],
                                F32,
                                tag="s",
                                bufs=4,
                                name=f"qkps_{part}_{sweep}_{ci}",
                            )
                            grp[(part, 2 * sweep + ci)] = g
                    else:
                        g = ps.tile(
                            [128, 2, CH],
                            F32,
                            tag="s",
                            bufs=2,
                            name=f"qkps_{part}_{sweep}",
                        )
                        for ci in (0, 1):
                            grp[(part, 2 * sweep + ci)] = g[:, ci, :]
                for ct in range(NCT):
                    for (part, c), g in grp.items():
                        cols = slice(128 * part, 128 * part + 128)
                        nc.tensor.matmul(
                            g,
                            w_sb[:, ct, cols],
                            x_sb[:, ct, CH * c : CH * c + CH],
                            start=(ct == 0),
                            stop=(ct == NCT - 1),
                        )
                for (part, c), g in grp.items():
                    dest = qT_sb if part == 0 else kT_sb
                    nc.vector.tensor_scalar_add(
                        dest[:, CH * c : CH * c + CH], g, bias_sb[:, part : part + 1]
                    )

            # v projection + PE transposes, per chunk; chunks 0/1 up front,
            # the rest emitted as PE filler inside stage D's j-loops.
            def emit_v_chunk(c):
                v_ps = ps.tile([128, CH], F32, tag="m", bufs=2, name=f"vps_{c}")
                for ct in range(NCT):
                    nc.tensor.matmul(
                        v_ps,
                        w_sb[:, ct, 256:384],
                        x_sb[:, ct, CH * c : CH * c + CH],
                        start=(ct == 0),
                        stop=(ct == NCT - 1),
                    )
                nc.vector.tensor_scalar_add(
                    vT_sb[:, CH * c : CH * c + CH], v_ps, bias_sb[:, 2:3]
                )

            def transposes_for(c):
                # one 128x128 transpose per k-tile covers both heads:
                # out cols 0-63 = head-0 dims, 64-127 = head-1 dims.
                for kt in range(4 * c, 4 * c + 4):
                    if "notr128" in _DBG:
                        for h, v_aug in ((0, v_aug0), (1, v_aug1)):
                            hrow = slice(64 * h, 64 * h + 64)
                            tr_ps = ps.tile(
                                [128, 64], BF16, tag="m", bufs=2, name=f"tr_{h}_{kt}"
                            )
                            nc.tensor.transpose(
                                tr_ps,
                                vT_sb[hrow, 128 * kt : 128 * kt + 128],
                                ident_sb[hrow, 0:64],
                            )
                            nc.vector.tensor_copy(v_aug[:, kt, 0:64], tr_ps)
                        continue
                    tr_ps = ps.tile([128, 128], BF16, tag="m", bufs=2, name=f"tr_{kt}")
                    nc.tensor.transpose(
                        tr_ps,
                        vT_sb[:, 128 * kt : 128 * kt + 128],
                        ident_sb,
                    )
                    nc.vector.tensor_copy(v_aug0[:, kt, 0:64], tr_ps[:, 0:64])
                    nc.vector.tensor_copy(v_aug1[:, kt, 0:64], tr_ps[:, 64:128])

            emit_v_chunk(0)
            transposes_for(0)
            emit_v_chunk(1)
            transposes_for(1)

            # ---- stages D+E per t-chunk ------------------------------------
            # Deferred work from chunk c-1, spread through chunk c's j-loop.
            pending_norm = None  # (pvA, pvB, chunk)
            pending_proj = None  # chunk index

            def emit_norm(pvA, pvB, pc):
                tcol = slice(CH * pc, CH * pc + CH)
                # denominators -> SBUF (f32r), both at partition 64
                nc.vector.tensor_copy(sums_sb[64:65, pc, 0, :], pvA[64:65, :])
                nc.vector.tensor_copy(sums_sb[64:65, pc, 1, :], pvB[64:65, :])
                # partition-broadcast via two K=1 matmuls
                rb0_ps = ps.tile([128, CH], F32, tag="m", bufs=2, name=f"rb0_{pc}")
                rb1_ps = ps.tile([128, CH], F32, tag="m", bufs=2, name=f"rb1_{pc}")
                nc.tensor.matmul(
                    rb0_ps,
                    ones_nrm[64:65, :],
                    sums_sb[64:65, pc, 0, :],
                    start=True,
                    stop=True,
                )
                nc.tensor.matmul(
                    rb1_ps,
                    ones_nrm[64:65, :],
                    sums_sb[64:65, pc, 1, :],
                    start=True,
                    stop=True,
                )
                nc.vector.reciprocal_approx_fast(
                    out=inv_sb[:, 0, :], in_=rb0_ps[0:64, :]
                )
                nc.vector.reciprocal_approx_fast(
                    out=inv_sb[:, 1, :], in_=rb1_ps[0:64, :]
                )
                nc.vector.tensor_mul(attn0[:, tcol], pvA[0:64, :], inv_sb[:, 0, :])
                nc.vector.tensor_mul(attn1[:, tcol], pvB[0:64, :], inv_sb[:, 1, :])

            def emit_proj_tile(pc, m, cast_eng=None):
                tcol = slice(CH * pc, CH * pc + CH)
                # Two K=64 matmuls at the same tile position accumulate into
                # one bank; same position forces serial execution, which is
                # what makes same-partition accumulation safe on HW.
                pr_ps = ps.tile([128, CH], F32, tag="m", bufs=2, name=f"pr_{m}_{pc}")
                nc.tensor.matmul(
                    pr_ps,
                    proj0_sb[:, 128 * m : 128 * m + 128],
                    attn0[:, tcol],
                    start=True,
                    stop=False,
                )
                nc.tensor.matmul(
                    pr_ps,
                    proj1_sb[:, 128 * m : 128 * m + 128],
                    attn1[:, tcol],
                    start=False,
                    stop=True,
                )
                ob = outev_pool.tile([128, CH], BF16, tag="outev", name=f"ob_{m}_{pc}")
                if cast_eng is nc.scalar and "noscalcast" not in _DBG:
                    nc.scalar.copy(ob, pr_ps)
                else:
                    nc.vector.tensor_copy(ob, pr_ps)
                nc.sync.dma_start(out=out[pc, m], in_=ob)

            # Chunk order (1, 2, 3, 0): the ScalarE-heavy late chunks get the
            # previous chunk's projection matmuls as PE filler, and the final
            # chunk processed (0) has the shortest tail.
            chunk_order = (1, 2, 3, 0)
            for f in _DBG:
                if f.startswith("c") and f[1:].isdigit():
                    chunk_order = (1, 2, 3, 0)[: int(f[1:])]
            for c in chunk_order:
                nj = 4 * c + 4
                pvA = ps.tile([128, CH], F32, tag="pv", bufs=2, name=f"pvA_{c}")
                pvB = ps.tile([128, CH], F32, tag="pv", bufs=2, name=f"pvB_{c}")
                pending = []
                proj_emitted = 0

                def emit_pv(item, last):
                    pj, pw, plo = item
                    # PSUM has_written clears are per-partition, so den1
                    # (bank B row 64) and pvB (rows 0:64) are independent
                    # accumulation groups in one bank; disjoint partitions
                    # make their concurrent drains safe.
                    nc.tensor.matmul(
                        pvA[0:65, plo:CH],
                        v_aug0[:, pj, :],
                        pw[:, 0, plo:CH],
                        start=(pj == 0),
                        stop=last,
                    )
                    nc.tensor.matmul(
                        pvB[0:64, plo:CH],
                        v_aug1[:, pj, :],
                        pw[:, 1, plo:CH],
                        start=(pj == 0),
                        stop=last,
                    )
                    # skip_group_check: the sim's group guard is per-bank but
                    # the HW clear/accumulate state is per-partition, and den1
                    # (row 64) is disjoint from pvB's rows 0:64.
                    nc.tensor.matmul(
                        pvB[64:65, plo:CH],
                        onesc_sb,
                        pw[:, 1, plo:CH],
                        start=(pj == 0),
                        stop=last,
                        skip_group_check=True,
                    )

                for j in range(nj):
                    if "nowides" in _DBG:
                        sa = ps.tile([128, CH], F32, tag="s", bufs=4, name=f"sa_{c}_{j}")
                        sb = ps.tile([128, CH], F32, tag="s", bufs=4, name=f"sb_{c}_{j}")
                        s_ps = _FakeWide((sa, sb))
                    else:
                        s_ps = ps.tile(
                            [128, 2, CH], F32, tag="s", bufs=2, name=f"s_{c}_{j}"
                        )
                    # diagonal tiles: columns < 128*diag are fully masked
                    # downstream, so don't compute their scores either
                    diag = j - 4 * c
                    slo = max(0, 128 * diag)
                    for h in (0, 1):
                        hrow = slice(64 * h, 64 * h + 64)
                        nc.tensor.matmul(
                            s_ps[:, h, slo:CH],
                            kT_sb[hrow, 128 * j : 128 * j + 128],
                            qT_sb[hrow, CH * c + slo : CH * c + CH],
                            start=True,
                            stop=True,
                        )
                    w_t = expw_pool.tile(
                        [128, 2, CH], BF16, tag="expw", name=f"w_{c}_{j}"
                    )
                    # one wide exp per j covers both heads (2 PSUM banks)
                    if "nowide" in _DBG or "nowides" in _DBG:
                        for h in (0, 1):
                            nc.scalar.activation(
                                out=w_t[:, h, slo:CH], in_=s_ps[:, h, slo:CH], func=EXP
                            )
                    else:
                        nc.scalar.activation(
                            out=w_t[:, :, slo:CH], in_=s_ps[:, :, slo:CH], func=EXP
                        )
                    if diag >= 0:
                        # keep exp(score) where t >= k: within the kept column
                        # range f' = f - 128*diag, so keep f' - p >= 0
                        for h in (0, 1):
                            nc.gpsimd.affine_select(
                                out=w_t[:, h, slo:CH],
                                in_=w_t[:, h, slo:CH],
                                pattern=[[1, CH - slo]],
                                compare_op=IS_GE,
                                fill=0.0,
                                base=0,
                                channel_multiplier=-1,
                            )
                    pending.append((j, w_t, slo))
                    if j == 0 and pending_norm is not None:
                        emit_norm(*pending_norm)
                        pending_norm = None
                    if j == 1 and c == 1:
                        emit_v_chunk(2)
                    if j == 2 and c == 1:
                        transposes_for(2)
                    if j == 1 and c == 3:
                        emit_v_chunk(3)
                    if j == 3 and c == 3:
                        transposes_for(3)
                    while len(pending) > PIPE:
                        item, pending = pending[0], pending[1:]
                        emit_pv(item, last=False)
                    if pending_proj is not None and j >= 1:
                        target = (j * 8) // max(nj - 1, 1)
                        while proj_emitted < min(target, 8):
                            emit_proj_tile(pending_proj, proj_emitted)
                            proj_emitted += 1
                while pending:
                    item, pending = pending[0], pending[1:]
                    emit_pv(item, last=(len(pending) == 0))
                if pending_proj is not None:
                    while proj_emitted < 8:
                        emit_proj_tile(pending_proj, proj_emitted)
                        proj_emitted += 1

                pending_norm = (pvA, pvB, c)
                pending_proj = c

            # tail: final chunk's norm + proj; casts alternate Scalar/Vector
            # (ScalarE is idle once the exps are done)
            if "notail" not in _DBG:
                emit_norm(*pending_norm)
                for m in range(8):
                    emit_proj_tile(
                        pending_proj, m, cast_eng=nc.scalar if m % 2 else None
                    )

    nc.compile()
    return nc


def _get_nc():
    if "nc" not in _CACHE:
        _CACHE["nc"] = _build()
    return _CACHE["nc"]


def _make_in_maps(x, wqkv_w, wqkv_b, proj_w):
    bf = ml_dtypes.bfloat16
    xT = np.ascontiguousarray(np.asarray(x, np.float32).T.astype(bf))
    identb = np.eye(128, dtype=bf)
    scale = np.float32(1.0 / np.sqrt(C))
    in_maps = []
    for i in range(N_CORES):
        rows = []
        biases = []
        for blk, s in ((0, scale), (1, None), (2, None)):
            sl = slice(blk * C + 128 * i, blk * C + 128 * i + 128)
            w = np.asarray(wqkv_w[sl], np.float32)
            b = np.asarray(wqkv_b[sl], np.float32)
            if s is not None:
                w = w * s
                b = b * s
            rows.append(w)
            biases.append(b)
        W = np.concatenate(rows, axis=0)  # [384, 1024]
        B = np.stack(biases, axis=1)  # [128, 3]
        pT = np.asarray(proj_w[:, 128 * i : 128 * i + 128], np.float32).T  # [128, 1024]
        normones = np.zeros((128, 128), np.float32)
        normones[0] = 1.0
        normones[64] = 1.0
        in_maps.append(
            {
                "xT": xT,
                "wqkv": np.ascontiguousarray(W.T.astype(bf)),
                "projT": np.ascontiguousarray(pT.astype(bf)),
                "identb": identb,
                "normones": normones,
                "bias": np.ascontiguousarray(B),
            }
        )
    return in_maps


def kernel(x, wqkv_w, wqkv_b, proj_w, proj_b, _trace=False, _tmpdir=None):
    from concourse.bass_utils import run_bass_kernel_spmd

    nc = _get_nc()
    in_maps = _make_in_maps(x, wqkv_w, wqkv_b, proj_w)
    res = run_bass_kernel_spmd(
        nc,
        in_maps,
        core_ids=list(range(N_CORES)),
        trace=_trace,
        tmpdir=_tmpdir,
    )
    acc = np.zeros((NT, 8, 128, CH), np.float64)
    for rmap in res.results:
        acc += rmap["out"].astype(np.float64)
    partialT = acc.transpose(1, 2, 0, 3).reshape(C, T)  # [o, t]
    full = partialT.T + np.asarray(proj_b, np.float64)[None, :]
    if _trace:
        _CACHE["last_result"] = res
    return full.astype(np.float32)


# revision 47
# speedup vs baseline: 1.0298x; 1.0240x over previous
"""Causal self-attention (T=2048, C=1024, H=16) on 8 trn2 NeuronCores.

Tensor-parallel over heads: core i computes heads 2i, 2i+1 (q/k/v rows
128i:128i+128 of each 1024-row block of wqkv_w, proj_w columns
128i:128i+128), producing a partial output projection; partials are summed
on the host (the all-reduce of the sharding hint).

Per-core Bass/Tile kernel, bf16 matmuls with fp32 PSUM accumulation.
Layout puts head 0's attention pipeline on partitions 0-63 and head 1's on
64-127 so the two heads' K=64 matmuls land in disjoint PE row groups
(auto tile_position from base partitions) and run concurrently:
  B. warmup matmuls on the identity tile keep HAM's activity window busy
     while the x DMAs stream, so stage B starts at K=8/8 (2.4GHz).
     qkvT[j, t] = wqkv.T @ xT, contraction-tile outer so matmuls chase the
     x DMAs; q rows pre-scaled by 1/sqrt(C) on the host. v's 128x128 PE
     transposes produce both heads' v_aug tiles at once and are interleaved
     with the v matmuls so they never form a transpose-only PE window.
  D. per 512-col t-chunk: sT[k, 2, t] = kT.T @ qT (both heads, one wide
     2-bank PSUM tile) -> ONE wide exp per j on ScalarE (bf16 out, no
     max-subtraction: |scores| < ~1) -> causal affine_select on gpsimd
     (diagonal k-tiles only, both heads in one 3D op) -> PV:
       pvA[0:65]  += v_aug0.T @ w0   (M=65, ones col = head-0 denominator)
       den1[96:97]+= ones.T   @ w1   (M=1 packed into PE col group 3,
                                      concurrent with the pvA matmul)
       pvB[64:128]+= v_aug1.T @ w1   (M=64 at base partition 64)
     Normalize on DVE: denominators broadcast via two concurrent K=1
     matmuls (row groups 2/3), reciprocal_approx_fast, one multiply per
     head -> attn_pair[128, T] bf16 (h0 rows 0-63, h1 rows 64-127).
     Previous chunk's normalize/proj spread through the j-loop as PE
     filler while ScalarE (exp) is the per-step bottleneck.
  E. partialT[o, t] = projT.T @ attn_pair: per o-tile, two concurrent
     K=64 matmuls (row groups {2,3} then {0,1}; the rows-64-127 one is
     emitted first / start=True because its drain path is shorter).
"""

import sys

if "/opt/trn_rl_repo" not in sys.path:
    sys.path.insert(0, "/opt/trn_rl_repo")

import os

import ml_dtypes
import numpy as np

_DBG = set(os.environ.get("KDBG", "").split(","))


class _FakeWide:
    """Pair of 1-bank PSUM tiles indexed like a [128, 2, CH] wide tile."""

    def __init__(self, tiles):
        self.tiles = tiles

    def __getitem__(self, idx):
        assert len(idx) == 3
        return self.tiles[idx[1]][idx[0], idx[2]]

T = 2048
C = 1024
CH = 512  # t-chunk width (one PSUM bank of fp32)
NT = T // CH  # 4 t-chunks
NK = T // 128  # 16 k-tiles
NCT = C // 128  # 8 contraction tiles
N_CORES = 8
PIPE = 3  # scores->PV pipeline depth in j-steps
N_WARM = 34  # warmup matmuls (N=128 cold ~107ns each => ~3.6us, one HAM window)

_CACHE = {}


def _build():
    import concourse.tile as tile
    from concourse import bacc, mybir

    F32 = mybir.dt.float32
    F32R = mybir.dt.float32r
    BF16 = mybir.dt.bfloat16
    EXP = mybir.ActivationFunctionType.Exp
    IS_GE = mybir.AluOpType.is_ge

    nc = bacc.Bacc(
        "TRN2",
        target_bir_lowering=False,
        debug=False,
        enable_asserts=False,
        num_devices=N_CORES,
        num_swdge_queues=4,
    )
    xT = nc.dram_tensor("xT", [C, T], BF16, kind="ExternalInput").ap()
    wqkv = nc.dram_tensor("wqkv", [C, 384], BF16, kind="ExternalInput").ap()
    projT = nc.dram_tensor("projT", [128, C], BF16, kind="ExternalInput").ap()
    identb = nc.dram_tensor("identb", [128, 128], BF16, kind="ExternalInput").ap()
    normones = nc.dram_tensor("normones", [128, 128], F32R, kind="ExternalInput").ap()
    bias = nc.dram_tensor("bias", [128, 3], F32, kind="ExternalInput").ap()
    # output as contiguous [chunk, o-tile, 128, 512] bf16 tiles: each store is
    # one fully-contiguous 128KB DMA
    out = nc.dram_tensor("out", [NT, 8, 128, CH], BF16, kind="ExternalOutput").ap()

    with tile.TileContext(nc) as tc:
        with (
            tc.tile_pool(name="big", bufs=1) as big,
            tc.tile_pool(name="expw", bufs=8) as expw_pool,
            tc.tile_pool(name="outev", bufs=3) as outev_pool,
            tc.tile_pool(name="ps", bufs=1, space="PSUM") as ps,
        ):
            # ---- resident SBUF tensors -------------------------------------
            x_sb = big.tile([128, NCT, T], BF16, name="x_sb")
            w_sb = big.tile([128, NCT, 384], BF16, name="w_sb")
            proj0_sb = big.tile([64, C], BF16, name="proj0_sb")
            proj1_sb = big.tile([64, C], BF16, name="proj1_sb")
            qT_sb = big.tile([128, T], BF16, name="qT_sb")
            kT_sb = big.tile([128, T], BF16, name="kT_sb")
            vT_sb = big.tile([128, T], BF16, name="vT_sb")
            v_aug0 = big.tile([128, NK, 65], BF16, name="v_aug0")
            v_aug1 = big.tile([128, NK, 64], BF16, name="v_aug1")
            attn0 = big.tile([64, T], BF16, name="attn0")
            attn1 = big.tile([64, T], BF16, name="attn1")
            ident_sb = big.tile([128, 128], BF16, name="ident_sb")
            onesc_sb = big.tile([128, 1], BF16, name="onesc_sb")
            ones_nrm = big.tile([128, 128], F32R, name="ones_nrm")
            sums_sb = big.tile([128, NT, 2, CH], F32R, name="sums_sb")
            inv_sb = big.tile([64, 2, CH], F32, name="inv_sb")
            bias_sb = big.tile([128, 3], F32, name="bias_sb")
            scr_sb = big.tile([1, 2], F32, name="scr_sb")

            # ident first: the warmup matmuls depend only on it.
            nc.sync.dma_start(out=ident_sb, in_=identb)
            nc.sync.dma_start(out=bias_sb, in_=bias)

            # warmup: keep the PE array busy through one HAM activity window
            # while the big x DMAs stream, so stage B starts at 2.4GHz.
            if "nowarm" not in _DBG:
                warm_ps = ps.tile([128, CH], F32, tag="m", bufs=2, name="warm_ps")
                for _ in range(N_WARM):
                    nc.tensor.matmul(
                        warm_ps[:, 0:128], ident_sb, ident_sb, start=True, stop=True
                    )
            # preload the exp activation table during the DMA ramp
            nc.scalar.activation(out=scr_sb[0:1, 0:1], in_=bias_sb[0:1, 0:1], func=EXP)

            # x and w tiles are fully-contiguous DRAM regions (full rows), so
            # each DMA streams at queue peak; three queues (scalar HWDGE ~2x
            # sync's rate, plus a gpsimd SWDGE queue) so sweep 1 is not
            # DMA-bound. w tiles go first on sync (needed per-ct immediately).
            x_q = {
                0: nc.scalar,
                1: nc.gpsimd,
                2: nc.scalar,
                3: nc.gpsimd,
                4: nc.scalar,
                5: nc.sync,
                6: nc.scalar,
                7: nc.sync,
            }
            for ct in range(NCT):
                nc.sync.dma_start(
                    out=w_sb[:, ct, :], in_=wqkv[128 * ct : 128 * ct + 128, :]
                )
                x_q[ct].dma_start(
                    out=x_sb[:, ct, :], in_=xT[128 * ct : 128 * ct + 128, :]
                )
            nc.sync.dma_start(out=proj0_sb, in_=projT[0:64, :])
            nc.sync.dma_start(out=proj1_sb, in_=projT[64:128, :])

            nc.sync.dma_start(out=ones_nrm, in_=normones)
            nc.vector.memset(v_aug0[:, :, 64:65], 1.0)
            nc.vector.memset(onesc_sb, 1.0)

            # ---- stage B: q/k projections ----------------------------------
            # Sweep 1 (q/k for chunks 0,1) runs up front, chasing the x DMAs;
            # sweep 2 is emitted in halves inside chunk 1's j-loop as PE
            # filler (each half holds only one wide s slot so the scores/exp
            # pipeline keeps the other).
            def qk_part(gs, part, cts, evac=False):
                # 1-bank m-tag groups so the scores/exp pipeline keeps both
                # wide s slots; emitted a few ct-steps per j as PE filler.
                cols = slice(128 * part, 128 * part + 128)
                for ct in cts:
                    for c, g in gs.items():
                        nc.tensor.matmul(
                            g,
                            w_sb[:, ct, cols],
                            x_sb[:, ct, CH * c : CH * c + CH],
                            start=(ct == 0),
                            stop=(ct == NCT - 1),
                        )
                if evac:
                    dest = qT_sb if part == 0 else kT_sb
                    for c, g in gs.items():
                        nc.vector.tensor_scalar_add(
                            dest[:, CH * c : CH * c + CH],
                            g,
                            bias_sb[:, part : part + 1],
                        )

            def qk_groups(tag_name):
                return {
                    c: ps.tile([128, CH], F32, tag="m", bufs=2, name=f"{tag_name}{c}")
                    for c in (2, 3)
                }

            # sweep 1: interleave q and k groups across both s slots, ct-outer
            # so the matmuls chase the x DMA stream.
            grp = {}
            for part in (0, 1):
                g = ps.tile([128, 2, CH], F32, tag="s", bufs=2, name=f"qkps_{part}")
                for ci in (0, 1):
                    grp[(part, ci)] = g[:, ci, :]
            for ct in range(NCT):
                for (part, c), g in grp.items():
                    cols = slice(128 * part, 128 * part + 128)
                    nc.tensor.matmul(
                        g,
                        w_sb[:, ct, cols],
                        x_sb[:, ct, CH * c : CH * c + CH],
                        start=(ct == 0),
                        stop=(ct == NCT - 1),
                    )
            for (part, c), g in grp.items():
                dest = qT_sb if part == 0 else kT_sb
                nc.vector.tensor_scalar_add(
                    dest[:, CH * c : CH * c + CH], g, bias_sb[:, part : part + 1]
                )

            # v projection + PE transposes, per chunk; chunks 0/1 up front,
            # the rest emitted as PE filler inside stage D's j-loops.
            def emit_v_chunk(c):
                v_ps = ps.tile([128, CH], F32, tag="m", bufs=2, name=f"vps_{c}")
                for ct in range(NCT):
                    nc.tensor.matmul(
                        v_ps,
                        w_sb[:, ct, 256:384],
                        x_sb[:, ct, CH * c : CH * c + CH],
                        start=(ct == 0),
                        stop=(ct == NCT - 1),
                    )
                nc.vector.tensor_scalar_add(
                    vT_sb[:, CH * c : CH * c + CH], v_ps, bias_sb[:, 2:3]
                )

            def transposes_for(c):
                # one 128x128 transpose per k-tile covers both heads:
                # out cols 0-63 = head-0 dims, 64-127 = head-1 dims.
                for kt in range(4 * c, 4 * c + 4):
                    if "notr128" in _DBG:
                        for h, v_aug in ((0, v_aug0), (1, v_aug1)):
                            hrow = slice(64 * h, 64 * h + 64)
                            tr_ps = ps.tile(
                                [128, 64], BF16, tag="m", bufs=2, name=f"tr_{h}_{kt}"
                            )
                            nc.tensor.transpose(
                                tr_ps,
                                vT_sb[hrow, 128 * kt : 128 * kt + 128],
                                ident_sb[hrow, 0:64],
                            )
                            nc.vector.tensor_copy(v_aug[:, kt, 0:64], tr_ps)
                        continue
                    tr_ps = ps.tile([128, 128], BF16, tag="m", bufs=2, name=f"tr_{kt}")
                    nc.tensor.transpose(
                        tr_ps,
                        vT_sb[:, 128 * kt : 128 * kt + 128],
                        ident_sb,
                    )
                    nc.vector.tensor_copy(v_aug0[:, kt, 0:64], tr_ps[:, 0:64])
                    nc.vector.tensor_copy(v_aug1[:, kt, 0:64], tr_ps[:, 64:128])

            emit_v_chunk(0)
            transposes_for(0)

            # ---- stages D+E per t-chunk ------------------------------------
            # Deferred work from chunk c-1, spread through chunk c's j-loop.
            pending_norm = None  # (pvA, pvB, chunk)
            pending_proj = None  # chunk index

            def emit_norm(pvA, pvB, pc):
                tcol = slice(CH * pc, CH * pc + CH)
                # denominators -> SBUF (f32r), both at partition 64
                nc.vector.tensor_copy(sums_sb[64:65, pc, 0, :], pvA[64:65, :])
                nc.vector.tensor_copy(sums_sb[64:65, pc, 1, :], pvB[64:65, :])
                # partition-broadcast via two K=1 matmuls
                rb0_ps = ps.tile([128, CH], F32, tag="m", bufs=2, name=f"rb0_{pc}")
                rb1_ps = ps.tile([128, CH], F32, tag="m", bufs=2, name=f"rb1_{pc}")
                nc.tensor.matmul(
                    rb0_ps,
                    ones_nrm[64:65, :],
                    sums_sb[64:65, pc, 0, :],
                    start=True,
                    stop=True,
                )
                nc.tensor.matmul(
                    rb1_ps,
                    ones_nrm[64:65, :],
                    sums_sb[64:65, pc, 1, :],
                    start=True,
                    stop=True,
                )
                nc.vector.reciprocal_approx_fast(
                    out=inv_sb[:, 0, :], in_=rb0_ps[0:64, :]
                )
                nc.vector.reciprocal_approx_fast(
                    out=inv_sb[:, 1, :], in_=rb1_ps[0:64, :]
                )
                nc.vector.tensor_mul(attn0[:, tcol], pvA[0:64, :], inv_sb[:, 0, :])
                nc.vector.tensor_mul(attn1[:, tcol], pvB[0:64, :], inv_sb[:, 1, :])

            def emit_proj_tile(pc, m, cast_eng=None):
                tcol = slice(CH * pc, CH * pc + CH)
                # Two K=64 matmuls at the same tile position accumulate into
                # one bank; same position forces serial execution, which is
                # what makes same-partition accumulation safe on HW.
                pr_ps = ps.tile([128, CH], F32, tag="m", bufs=2, name=f"pr_{m}_{pc}")
                nc.tensor.matmul(
                    pr_ps,
                    proj0_sb[:, 128 * m : 128 * m + 128],
                    attn0[:, tcol],
                    start=True,
                    stop=False,
                )
                nc.tensor.matmul(
                    pr_ps,
                    proj1_sb[:, 128 * m : 128 * m + 128],
                    attn1[:, tcol],
                    start=False,
                    stop=True,
                )
                ob = outev_pool.tile([128, CH], BF16, tag="outev", name=f"ob_{m}_{pc}")
                if cast_eng is nc.scalar and "noscalcast" not in _DBG:
                    nc.scalar.copy(ob, pr_ps)
                else:
                    nc.vector.tensor_copy(ob, pr_ps)
                nc.sync.dma_start(out=out[pc, m], in_=ob)

            # Chunk order (1, 2, 3, 0): the ScalarE-heavy late chunks get the
            # previous chunk's projection matmuls as PE filler, and the final
            # chunk processed (0) has the shortest tail.
            chunk_order = (1, 2, 3, 0)
            for f in _DBG:
                if f.startswith("c") and f[1:].isdigit():
                    chunk_order = (1, 2, 3, 0)[: int(f[1:])]
            for c in chunk_order:
                nj = 4 * c + 4
                pvA = ps.tile([128, CH], F32, tag="pv", bufs=2, name=f"pvA_{c}")
                pvB = ps.tile([128, CH], F32, tag="pv", bufs=2, name=f"pvB_{c}")
                pending = []
                proj_emitted = 0

                def emit_pv(item, last):
                    pj, pw, plo = item
                    # PSUM has_written clears are per-partition, so den1
                    # (bank B row 64) and pvB (rows 0:64) are independent
                    # accumulation groups in one bank; disjoint partitions
                    # make their concurrent drains safe.
                    nc.tensor.matmul(
                        pvA[0:65, plo:CH],
                        v_aug0[:, pj, :],
                        pw[:, 0, plo:CH],
                        start=(pj == 0),
                        stop=last,
                    )
                    nc.tensor.matmul(
                        pvB[0:64, plo:CH],
                        v_aug1[:, pj, :],
                        pw[:, 1, plo:CH],
                        start=(pj == 0),
                        stop=last,
                    )
                    # skip_group_check: the sim's group guard is per-bank but
                    # the HW clear/accumulate state is per-partition, and den1
                    # (row 64) is disjoint from pvB's rows 0:64.
                    nc.tensor.matmul(
                        pvB[64:65, plo:CH],
                        onesc_sb,
                        pw[:, 1, plo:CH],
                        start=(pj == 0),
                        stop=last,
                        skip_group_check=True,
                    )

                for j in range(nj):
                    if "nowides" in _DBG:
                        sa = ps.tile([128, CH], F32, tag="s", bufs=4, name=f"sa_{c}_{j}")
                        sb = ps.tile([128, CH], F32, tag="s", bufs=4, name=f"sb_{c}_{j}")
                        s_ps = _FakeWide((sa, sb))
                    else:
                        s_ps = ps.tile(
                            [128, 2, CH], F32, tag="s", bufs=2, name=f"s_{c}_{j}"
                        )
                    # diagonal tiles: columns < 128*diag are fully masked
                    # downstream, so don't compute their scores either
                    diag = j - 4 * c
                    slo = max(0, 128 * diag)
                    for h in (0, 1):
                        hrow = slice(64 * h, 64 * h + 64)
                        nc.tensor.matmul(
                            s_ps[:, h, slo:CH],
                            kT_sb[hrow, 128 * j : 128 * j + 128],
                            qT_sb[hrow, CH * c + slo : CH * c + CH],
                            start=True,
                            stop=True,
                        )
                    w_t = expw_pool.tile(
                        [128, 2, CH], BF16, tag="expw", name=f"w_{c}_{j}"
                    )
                    # one wide exp per j covers both heads (2 PSUM banks)
                    if "nowide" in _DBG or "nowides" in _DBG:
                        for h in (0, 1):
                            nc.scalar.activation(
                                out=w_t[:, h, slo:CH], in_=s_ps[:, h, slo:CH], func=EXP
                            )
                    else:
                        nc.scalar.activation(
                            out=w_t[:, :, slo:CH], in_=s_ps[:, :, slo:CH], func=EXP
                        )
                    if diag >= 0:
                        # keep exp(score) where t >= k: within the kept column
                        # range f' = f - 128*diag, so keep f' - p >= 0
                        for h in (0, 1):
                            nc.gpsimd.affine_select(
                                out=w_t[:, h, slo:CH],
                                in_=w_t[:, h, slo:CH],
                                pattern=[[1, CH - slo]],
                                compare_op=IS_GE,
                                fill=0.0,
                                base=0,
                                channel_multiplier=-1,
                            )
                    pending.append((j, w_t, slo))
                    if j == 0 and pending_norm is not None:
                        emit_norm(*pending_norm)
                        pending_norm = None
                    if c == 1:
                        # sweep 2 (q/k chunks 2,3) + v1 as chunk 1's filler,
                        # a few hundred ns of PE work per j-step
                        if j == 0:
                            qk2q = qk_groups("qk2q")
                            qk_part(qk2q, 0, range(0, 4))
                        if j == 1:
                            qk_part(qk2q, 0, range(4, 8), evac=True)
                        if j == 2:
                            qk2k = qk_groups("qk2k")
                            qk_part(qk2k, 1, range(0, 4))
                        if j == 3:
                            qk_part(qk2k, 1, range(4, 8), evac=True)
                        if j == 4:
                            emit_v_chunk(1)
                        if j == 5:
                            transposes_for(1)
                    if j == 1 and c == 2:
                        emit_v_chunk(2)
                    if j == 2 and c == 2:
                        transposes_for(2)
                    if j == 1 and c == 3:
                        emit_v_chunk(3)
                    if j == 3 and c == 3:
                        transposes_for(3)
                    while len(pending) > PIPE:
                        item, pending = pending[0], pending[1:]
                        emit_pv(item, last=False)
                    if pending_proj is not None and j >= 1:
                        target = (j * 8) // max(nj - 1, 1)
                        while proj_emitted < min(target, 8):
                            emit_proj_tile(pending_proj, proj_emitted)
                            proj_emitted += 1
                while pending:
                    item, pending = pending[0], pending[1:]
                    emit_pv(item, last=(len(pending) == 0))
                if pending_proj is not None:
                    while proj_emitted < 8:
                        emit_proj_tile(pending_proj, proj_emitted)
                        proj_emitted += 1

                pending_norm = (pvA, pvB, c)
                pending_proj = c

            # tail: final chunk's norm + proj; casts alternate Scalar/Vector
            # (ScalarE is idle once the exps are done)
            if "notail" not in _DBG:
                emit_norm(*pending_norm)
                for m in range(8):
                    emit_proj_tile(
                        pending_proj, m, cast_eng=nc.scalar if m % 2 else None
                    )

    nc.compile()
    return nc


def _get_nc():
    if "nc" not in _CACHE:
        _CACHE["nc"] = _build()
    return _CACHE["nc"]


def _make_in_maps(x, wqkv_w, wqkv_b, proj_w):
    bf = ml_dtypes.bfloat16
    xT = np.ascontiguousarray(np.asarray(x, np.float32).T.astype(bf))
    identb = np.eye(128, dtype=bf)
    scale = np.float32(1.0 / np.sqrt(C))
    in_maps = []
    for i in range(N_CORES):
        rows = []
        biases = []
        for blk, s in ((0, scale), (1, None), (2, None)):
            sl = slice(blk * C + 128 * i, blk * C + 128 * i + 128)
            w = np.asarray(wqkv_w[sl], np.float32)
            b = np.asarray(wqkv_b[sl], np.float32)
            if s is not None:
                w = w * s
                b = b * s
            rows.append(w)
            biases.append(b)
        W = np.concatenate(rows, axis=0)  # [384, 1024]
        B = np.stack(biases, axis=1)  # [128, 3]
        pT = np.asarray(proj_w[:, 128 * i : 128 * i + 128], np.float32).T  # [128, 1024]
        normones = np.zeros((128, 128), np.float32)
        normones[0] = 1.0
        normones[64] = 1.0
        in_maps.append(
            {
                "xT": xT,
                "wqkv": np.ascontiguousarray(W.T.astype(bf)),
                "projT": np.ascontiguousarray(pT.astype(bf)),
                "identb": identb,
                "normones": normones,
                "bias": np.ascontiguousarray(B),
            }
        )
    return in_maps


def kernel(x, wqkv_w, wqkv_b, proj_w, proj_b, _trace=False, _tmpdir=None):
    from concourse.bass_utils import run_bass_kernel_spmd

    nc = _get_nc()
    in_maps = _make_in_maps(x, wqkv_w, wqkv_b, proj_w)
    res = run_bass_kernel_spmd(
        nc,
        in_maps,
        core_ids=list(range(N_CORES)),
        trace=_trace,
        tmpdir=_tmpdir,
    )
    acc = np.zeros((NT, 8, 128, CH), np.float64)
    for rmap in res.results:
        acc += rmap["out"].astype(np.float64)
    partialT = acc.transpose(1, 2, 0, 3).reshape(C, T)  # [o, t]
    full = partialT.T + np.asarray(proj_b, np.float64)[None, :]
    if _trace:
        _CACHE["last_result"] = res
    return full.astype(np.float32)


# revision 50
# speedup vs baseline: 1.0757x; 1.0446x over previous
"""Causal self-attention (T=2048, C=1024, H=16) on 8 trn2 NeuronCores.

Tensor-parallel over heads: core i computes heads 2i, 2i+1 (q/k/v rows
128i:128i+128 of each 1024-row block of wqkv_w, proj_w columns
128i:128i+128), producing a partial output projection; partials are summed
on the host (the all-reduce of the sharding hint).

Per-core Bass/Tile kernel, bf16 matmuls with fp32 PSUM accumulation.
Layout puts head 0's attention pipeline on partitions 0-63 and head 1's on
64-127 so the two heads' K=64 matmuls land in disjoint PE row groups
(auto tile_position from base partitions) and run concurrently:
  B. warmup matmuls on the identity tile keep HAM's activity window busy
     while the x DMAs stream, so stage B starts at K=8/8 (2.4GHz).
     qkvT[j, t] = wqkv.T @ xT, contraction-tile outer so matmuls chase the
     x DMAs; q rows pre-scaled by 1/sqrt(C) on the host. v's 128x128 PE
     transposes produce both heads' v_aug tiles at once and are interleaved
     with the v matmuls so they never form a transpose-only PE window.
  D. per 512-col t-chunk: sT[k, 2, t] = kT.T @ qT (both heads, one wide
     2-bank PSUM tile) -> ONE wide exp per j on ScalarE (bf16 out, no
     max-subtraction: |scores| < ~1) -> causal affine_select on gpsimd
     (diagonal k-tiles only, both heads in one 3D op) -> PV:
       pvA[0:65]  += v_aug0.T @ w0   (M=65, ones col = head-0 denominator)
       den1[96:97]+= ones.T   @ w1   (M=1 packed into PE col group 3,
                                      concurrent with the pvA matmul)
       pvB[64:128]+= v_aug1.T @ w1   (M=64 at base partition 64)
     Normalize on DVE: denominators broadcast via two concurrent K=1
     matmuls (row groups 2/3), reciprocal_approx_fast, one multiply per
     head -> attn_pair[128, T] bf16 (h0 rows 0-63, h1 rows 64-127).
     Previous chunk's normalize/proj spread through the j-loop as PE
     filler while ScalarE (exp) is the per-step bottleneck.
  E. partialT[o, t] = projT.T @ attn_pair: per o-tile, two concurrent
     K=64 matmuls (row groups {2,3} then {0,1}; the rows-64-127 one is
     emitted first / start=True because its drain path is shorter).
"""

import sys

if "/opt/trn_rl_repo" not in sys.path:
    sys.path.insert(0, "/opt/trn_rl_repo")

import os

import ml_dtypes
import numpy as np

_DBG = set(os.environ.get("KDBG", "").split(","))


class _FakeWide:
    """Pair of 1-bank PSUM tiles indexed like a [128, 2, CH] wide tile."""

    def __init__(self, tiles):
        self.tiles = tiles

    def __getitem__(self, idx):
        assert len(idx) == 3
        return self.tiles[idx[1]][idx[0], idx[2]]

T = 2048
C = 1024
CH = 512  # t-chunk width (one PSUM bank of fp32)
NT = T // CH  # 4 t-chunks
NK = T // 128  # 16 k-tiles
NCT = C // 128  # 8 contraction tiles
N_CORES = 8
PIPE = 3  # scores->PV pipeline depth in j-steps
N_WARM = 34  # warmup matmuls (N=128 cold ~107ns each => ~3.6us, one HAM window)

_CACHE = {}


def _build():
    import concourse.tile as tile
    from concourse import bacc, mybir

    F32 = mybir.dt.float32
    F32R = mybir.dt.float32r
    BF16 = mybir.dt.bfloat16
    EXP = mybir.ActivationFunctionType.Exp
    IS_GE = mybir.AluOpType.is_ge

    nc = bacc.Bacc(
        "TRN2",
        target_bir_lowering=False,
        debug=False,
        enable_asserts=False,
        num_devices=N_CORES,
        num_swdge_queues=4,
    )
    xT = nc.dram_tensor("xT", [C, T], BF16, kind="ExternalInput").ap()
    wqkv = nc.dram_tensor("wqkv", [C, 384], BF16, kind="ExternalInput").ap()
    projT = nc.dram_tensor("projT", [128, C], BF16, kind="ExternalInput").ap()
    identb = nc.dram_tensor("identb", [128, 128], BF16, kind="ExternalInput").ap()
    normones = nc.dram_tensor("normones", [128, 128], F32R, kind="ExternalInput").ap()
    bias = nc.dram_tensor("bias", [128, 3], F32, kind="ExternalInput").ap()
    # output as contiguous [chunk, o-tile, 128, 512] bf16 tiles: each store is
    # one fully-contiguous 128KB DMA
    out = nc.dram_tensor("out", [NT, 8, 128, CH], BF16, kind="ExternalOutput").ap()

    with tile.TileContext(nc) as tc:
        with (
            tc.tile_pool(name="big", bufs=1) as big,
            tc.tile_pool(name="expw", bufs=8) as expw_pool,
            tc.tile_pool(name="outev", bufs=3) as outev_pool,
            tc.tile_pool(name="ps", bufs=1, space="PSUM") as ps,
        ):
            # ---- resident SBUF tensors -------------------------------------
            x_sb = big.tile([128, NCT, T], BF16, name="x_sb")
            w_sb = big.tile([128, NCT, 384], BF16, name="w_sb")
            proj0_sb = big.tile([64, C], BF16, name="proj0_sb")
            proj1_sb = big.tile([64, C], BF16, name="proj1_sb")
            qT_sb = big.tile([128, T], BF16, name="qT_sb")
            kT_sb = big.tile([128, T], BF16, name="kT_sb")
            vT_sb = big.tile([128, T], BF16, name="vT_sb")
            v_aug0 = big.tile([128, NK, 65], BF16, name="v_aug0")
            v_aug1 = big.tile([128, NK, 64], BF16, name="v_aug1")
            attn0 = big.tile([64, T], BF16, name="attn0")
            attn1 = big.tile([64, T], BF16, name="attn1")
            ident_sb = big.tile([128, 128], BF16, name="ident_sb")
            onesc_sb = big.tile([128, 1], BF16, name="onesc_sb")
            ones_nrm = big.tile([128, 128], F32R, name="ones_nrm")
            sums_sb = big.tile([128, NT, 2, CH], F32R, name="sums_sb")
            inv_sb = big.tile([64, 2, CH], F32, name="inv_sb")
            bias_sb = big.tile([128, 3], F32, name="bias_sb")
            scr_sb = big.tile([1, 2], F32, name="scr_sb")

            # ident first: the warmup matmuls and the exp-table preload
            # depend only on it (depending on a later DMA would stall the
            # issuing queue).
            nc.sync.dma_start(out=ident_sb, in_=identb)
            nc.sync.dma_start(out=bias_sb, in_=bias)
            warm_ps = ps.tile([128, CH], F32, tag="m", bufs=2, name="warm_ps")

            def warm(n):
                if "nowarm" in _DBG:
                    return
                for _ in range(n):
                    nc.tensor.matmul(
                        warm_ps[:, 0:128], ident_sb, ident_sb, start=True, stop=True
                    )

            # preload the exp activation table during the DMA ramp
            nc.scalar.activation(
                out=scr_sb[0:1, 0:1], in_=ident_sb[0:1, 0:1], func=EXP
            )

            # x is loaded in column halves: sweep 1 (q/k chunks 0,1) and
            # v0/v1 only touch cols 0:1024, so the critical input load is
            # w + half of x. Three queues (sync/scalar HWDGE, gpsimd SWDGE).
            for ct in range(NCT):
                nc.sync.dma_start(
                    out=w_sb[:, ct, :], in_=wqkv[128 * ct : 128 * ct + 128, :]
                )
                xeng = nc.scalar if ct < 4 else nc.gpsimd
                xeng.dma_start(
                    out=x_sb[:, ct, 0:1024],
                    in_=xT[128 * ct : 128 * ct + 128, 0:1024],
                )
            for ct in range(NCT):
                xeng = (nc.sync, nc.scalar, nc.gpsimd)[(0, 0, 0, 0, 1, 1, 2, 2)[ct]]
                xeng.dma_start(
                    out=x_sb[:, ct, 1024:2048],
                    in_=xT[128 * ct : 128 * ct + 128, 1024:2048],
                )
            nc.sync.dma_start(out=proj0_sb, in_=projT[0:64, :])
            nc.sync.dma_start(out=proj1_sb, in_=projT[64:128, :])

            nc.sync.dma_start(out=ones_nrm, in_=normones)
            nc.vector.memset(v_aug0[:, :, 64:65], 1.0)
            nc.vector.memset(onesc_sb, 1.0)

            # ---- stage B: q/k projections ----------------------------------
            # Sweep 1 (q/k for chunks 0,1) runs up front, chasing the x DMAs;
            # sweep 2 is emitted in halves inside chunk 1's j-loop as PE
            # filler (each half holds only one wide s slot so the scores/exp
            # pipeline keeps the other).
            def qk_part(gs, part, cts, evac=False):
                # 1-bank m-tag groups so the scores/exp pipeline keeps both
                # wide s slots; emitted a few ct-steps per j as PE filler.
                cols = slice(128 * part, 128 * part + 128)
                for ct in cts:
                    for c, g in gs.items():
                        nc.tensor.matmul(
                            g,
                            w_sb[:, ct, cols],
                            x_sb[:, ct, CH * c : CH * c + CH],
                            start=(ct == 0),
                            stop=(ct == NCT - 1),
                        )
                if evac:
                    dest = qT_sb if part == 0 else kT_sb
                    for c, g in gs.items():
                        nc.vector.tensor_scalar_add(
                            dest[:, CH * c : CH * c + CH],
                            g,
                            bias_sb[:, part : part + 1],
                        )

            def qk_groups(tag_name):
                return {
                    c: ps.tile([128, CH], F32, tag="m", bufs=2, name=f"{tag_name}{c}")
                    for c in (2, 3)
                }

            # sweep 1: interleave q and k groups across both s slots, ct-outer
            # so the matmuls chase the x DMA stream; warmup matmuls between
            # ct groups keep the HAM activity window busy through DMA gaps.
            warm(8)
            grp = {}
            for part in (0, 1):
                g = ps.tile([128, 2, CH], F32, tag="s", bufs=2, name=f"qkps_{part}")
                for ci in (0, 1):
                    grp[(part, ci)] = g[:, ci, :]
            for ct in range(NCT):
                for (part, c), g in grp.items():
                    cols = slice(128 * part, 128 * part + 128)
                    nc.tensor.matmul(
                        g,
                        w_sb[:, ct, cols],
                        x_sb[:, ct, CH * c : CH * c + CH],
                        start=(ct == 0),
                        stop=(ct == NCT - 1),
                    )
                if ct < NCT - 1:
                    warm(4)
            for (part, c), g in grp.items():
                dest = qT_sb if part == 0 else kT_sb
                nc.vector.tensor_scalar_add(
                    dest[:, CH * c : CH * c + CH], g, bias_sb[:, part : part + 1]
                )

            # v projection + PE transposes, per chunk; chunks 0/1 up front,
            # the rest emitted as PE filler inside stage D's j-loops.
            def emit_v_chunk(c):
                v_ps = ps.tile([128, CH], F32, tag="m", bufs=2, name=f"vps_{c}")
                for ct in range(NCT):
                    nc.tensor.matmul(
                        v_ps,
                        w_sb[:, ct, 256:384],
                        x_sb[:, ct, CH * c : CH * c + CH],
                        start=(ct == 0),
                        stop=(ct == NCT - 1),
                    )
                nc.vector.tensor_scalar_add(
                    vT_sb[:, CH * c : CH * c + CH], v_ps, bias_sb[:, 2:3]
                )

            def transposes_for(c):
                # one 128x128 transpose per k-tile covers both heads:
                # out cols 0-63 = head-0 dims, 64-127 = head-1 dims.
                for kt in range(4 * c, 4 * c + 4):
                    if "notr128" in _DBG:
                        for h, v_aug in ((0, v_aug0), (1, v_aug1)):
                            hrow = slice(64 * h, 64 * h + 64)
                            tr_ps = ps.tile(
                                [128, 64], BF16, tag="m", bufs=2, name=f"tr_{h}_{kt}"
                            )
                            nc.tensor.transpose(
                                tr_ps,
                                vT_sb[hrow, 128 * kt : 128 * kt + 128],
                                ident_sb[hrow, 0:64],
                            )
                            nc.vector.tensor_copy(v_aug[:, kt, 0:64], tr_ps)
                        continue
                    tr_ps = ps.tile([128, 128], BF16, tag="m", bufs=2, name=f"tr_{kt}")
                    nc.tensor.transpose(
                        tr_ps,
                        vT_sb[:, 128 * kt : 128 * kt + 128],
                        ident_sb,
                    )
                    nc.vector.tensor_copy(v_aug0[:, kt, 0:64], tr_ps[:, 0:64])
                    nc.vector.tensor_copy(v_aug1[:, kt, 0:64], tr_ps[:, 64:128])

            emit_v_chunk(0)
            transposes_for(0)

            # ---- stages D+E per t-chunk ------------------------------------
            # Deferred work from chunk c-1, spread through chunk c's j-loop.
            pending_norm = None  # (pvA, pvB, chunk)
            pending_proj = None  # chunk index

            def emit_norm(pvA, pvB, pc):
                tcol = slice(CH * pc, CH * pc + CH)
                # denominators -> SBUF (f32r), both at partition 64
                nc.vector.tensor_copy(sums_sb[64:65, pc, 0, :], pvA[64:65, :])
                nc.vector.tensor_copy(sums_sb[64:65, pc, 1, :], pvB[64:65, :])
                # partition-broadcast via two K=1 matmuls
                rb0_ps = ps.tile([128, CH], F32, tag="m", bufs=2, name=f"rb0_{pc}")
                rb1_ps = ps.tile([128, CH], F32, tag="m", bufs=2, name=f"rb1_{pc}")
                nc.tensor.matmul(
                    rb0_ps,
                    ones_nrm[64:65, :],
                    sums_sb[64:65, pc, 0, :],
                    start=True,
                    stop=True,
                )
                nc.tensor.matmul(
                    rb1_ps,
                    ones_nrm[64:65, :],
                    sums_sb[64:65, pc, 1, :],
                    start=True,
                    stop=True,
                )
                nc.vector.reciprocal_approx_fast(
                    out=inv_sb[:, 0, :], in_=rb0_ps[0:64, :]
                )
                nc.vector.reciprocal_approx_fast(
                    out=inv_sb[:, 1, :], in_=rb1_ps[0:64, :]
                )
                nc.vector.tensor_mul(attn0[:, tcol], pvA[0:64, :], inv_sb[:, 0, :])
                nc.vector.tensor_mul(attn1[:, tcol], pvB[0:64, :], inv_sb[:, 1, :])

            def emit_proj_tile(pc, m, cast_eng=None):
                tcol = slice(CH * pc, CH * pc + CH)
                # Two K=64 matmuls at the same tile position accumulate into
                # one bank; same position forces serial execution, which is
                # what makes same-partition accumulation safe on HW.
                pr_ps = ps.tile([128, CH], F32, tag="m", bufs=2, name=f"pr_{m}_{pc}")
                nc.tensor.matmul(
                    pr_ps,
                    proj0_sb[:, 128 * m : 128 * m + 128],
                    attn0[:, tcol],
                    start=True,
                    stop=False,
                )
                nc.tensor.matmul(
                    pr_ps,
                    proj1_sb[:, 128 * m : 128 * m + 128],
                    attn1[:, tcol],
                    start=False,
                    stop=True,
                )
                ob = outev_pool.tile([128, CH], BF16, tag="outev", name=f"ob_{m}_{pc}")
                if cast_eng is nc.scalar and "noscalcast" not in _DBG:
                    nc.scalar.copy(ob, pr_ps)
                else:
                    nc.vector.tensor_copy(ob, pr_ps)
                nc.sync.dma_start(out=out[pc, m], in_=ob)

            # Chunk order (1, 2, 3, 0): the ScalarE-heavy late chunks get the
            # previous chunk's projection matmuls as PE filler, and the final
            # chunk processed (0) has the shortest tail.
            chunk_order = (1, 2, 3, 0)
            for f in _DBG:
                if f.startswith("c") and f[1:].isdigit():
                    chunk_order = (1, 2, 3, 0)[: int(f[1:])]
            for c in chunk_order:
                nj = 4 * c + 4
                pvA = ps.tile([128, CH], F32, tag="pv", bufs=2, name=f"pvA_{c}")
                pvB = ps.tile([128, CH], F32, tag="pv", bufs=2, name=f"pvB_{c}")
                pending = []
                proj_emitted = 0

                def emit_pv(item, last):
                    pj, pw, plo = item
                    # PSUM has_written clears are per-partition, so den1
                    # (bank B row 64) and pvB (rows 0:64) are independent
                    # accumulation groups in one bank; disjoint partitions
                    # make their concurrent drains safe.
                    nc.tensor.matmul(
                        pvA[0:65, plo:CH],
                        v_aug0[:, pj, :],
                        pw[:, 0, plo:CH],
                        start=(pj == 0),
                        stop=last,
                    )
                    nc.tensor.matmul(
                        pvB[0:64, plo:CH],
                        v_aug1[:, pj, :],
                        pw[:, 1, plo:CH],
                        start=(pj == 0),
                        stop=last,
                    )
                    # skip_group_check: the sim's group guard is per-bank but
                    # the HW clear/accumulate state is per-partition, and den1
                    # (row 64) is disjoint from pvB's rows 0:64.
                    nc.tensor.matmul(
                        pvB[64:65, plo:CH],
                        onesc_sb,
                        pw[:, 1, plo:CH],
                        start=(pj == 0),
                        stop=last,
                        skip_group_check=True,
                    )

                for j in range(nj):
                    if "nowides" in _DBG:
                        sa = ps.tile([128, CH], F32, tag="s", bufs=4, name=f"sa_{c}_{j}")
                        sb = ps.tile([128, CH], F32, tag="s", bufs=4, name=f"sb_{c}_{j}")
                        s_ps = _FakeWide((sa, sb))
                    else:
                        s_ps = ps.tile(
                            [128, 2, CH], F32, tag="s", bufs=2, name=f"s_{c}_{j}"
                        )
                    # diagonal tiles: columns < 128*diag are fully masked
                    # downstream, so don't compute their scores either
                    diag = j - 4 * c
                    slo = max(0, 128 * diag)
                    for h in (0, 1):
                        hrow = slice(64 * h, 64 * h + 64)
                        nc.tensor.matmul(
                            s_ps[:, h, slo:CH],
                            kT_sb[hrow, 128 * j : 128 * j + 128],
                            qT_sb[hrow, CH * c + slo : CH * c + CH],
                            start=True,
                            stop=True,
                        )
                    w_t = expw_pool.tile(
                        [128, 2, CH], BF16, tag="expw", name=f"w_{c}_{j}"
                    )
                    # one wide exp per j covers both heads (2 PSUM banks)
                    if "nowide" in _DBG or "nowides" in _DBG:
                        for h in (0, 1):
                            nc.scalar.activation(
                                out=w_t[:, h, slo:CH], in_=s_ps[:, h, slo:CH], func=EXP
                            )
                    else:
                        nc.scalar.activation(
                            out=w_t[:, :, slo:CH], in_=s_ps[:, :, slo:CH], func=EXP
                        )
                    if diag >= 0:
                        # keep exp(score) where t >= k: within the kept column
                        # range f' = f - 128*diag, so keep f' - p >= 0
                        for h in (0, 1):
                            nc.gpsimd.affine_select(
                                out=w_t[:, h, slo:CH],
                                in_=w_t[:, h, slo:CH],
                                pattern=[[1, CH - slo]],
                                compare_op=IS_GE,
                                fill=0.0,
                                base=0,
                                channel_multiplier=-1,
                            )
                    pending.append((j, w_t, slo))
                    if j == 0 and pending_norm is not None:
                        emit_norm(*pending_norm)
                        pending_norm = None
                    if c == 1:
                        # sweep 2 (q/k chunks 2,3) + v1 as chunk 1's filler,
                        # a few hundred ns of PE work per j-step
                        if j == 0:
                            qk2q = qk_groups("qk2q")
                            qk_part(qk2q, 0, range(0, 4))
                        if j == 1:
                            qk_part(qk2q, 0, range(4, 8), evac=True)
                        if j == 2:
                            qk2k = qk_groups("qk2k")
                            qk_part(qk2k, 1, range(0, 4))
                        if j == 3:
                            qk_part(qk2k, 1, range(4, 8), evac=True)
                        if j == 4:
                            emit_v_chunk(1)
                        if j == 5:
                            transposes_for(1)
                    if j == 1 and c == 2:
                        emit_v_chunk(2)
                    if j == 2 and c == 2:
                        transposes_for(2)
                    if j == 1 and c == 3:
                        emit_v_chunk(3)
                    if j == 3 and c == 3:
                        transposes_for(3)
                    while len(pending) > PIPE:
                        item, pending = pending[0], pending[1:]
                        emit_pv(item, last=False)
                    if pending_proj is not None and j >= 1:
                        target = (j * 8) // max(nj - 1, 1)
                        while proj_emitted < min(target, 8):
                            # in the final (short) chunk ScalarE has slack:
                            # let it take half the evacuation casts
                            ce = nc.scalar if (c == 0 and proj_emitted % 2) else None
                            emit_proj_tile(pending_proj, proj_emitted, cast_eng=ce)
                            proj_emitted += 1
                while pending:
                    item, pending = pending[0], pending[1:]
                    emit_pv(item, last=(len(pending) == 0))
                if pending_proj is not None:
                    while proj_emitted < 8:
                        emit_proj_tile(pending_proj, proj_emitted)
                        proj_emitted += 1

                pending_norm = (pvA, pvB, c)
                pending_proj = c

            # tail: final chunk's norm + proj; casts alternate Scalar/Vector
            # (ScalarE is idle once the exps are done)
            if "notail" not in _DBG:
                emit_norm(*pending_norm)
                for m in range(8):
                    emit_proj_tile(
                        pending_proj, m, cast_eng=nc.scalar if m % 2 else None
                    )

    nc.compile()
    return nc


def _get_nc():
    if "nc" not in _CACHE:
        _CACHE["nc"] = _build()
    return _CACHE["nc"]


def _make_in_maps(x, wqkv_w, wqkv_b, proj_w):
    bf = ml_dtypes.bfloat16
    xT = np.ascontiguousarray(np.asarray(x, np.float32).T.astype(bf))
    identb = np.eye(128, dtype=bf)
    scale = np.float32(1.0 / np.sqrt(C))
    in_maps = []
    for i in range(N_CORES):
        rows = []
        biases = []
        for blk, s in ((0, scale), (1, None), (2, None)):
            sl = slice(blk * C + 128 * i, blk * C + 128 * i + 128)
            w = np.asarray(wqkv_w[sl], np.float32)
            b = np.asarray(wqkv_b[sl], np.float32)
            if s is not None:
                w = w * s
                b = b * s
            rows.append(w)
            biases.append(b)
        W = np.concatenate(rows, axis=0)  # [384, 1024]
        B = np.stack(biases, axis=1)  # [128, 3]
        pT = np.asarray(proj_w[:, 128 * i : 128 * i + 128], np.float32).T  # [128, 1024]
        normones = np.zeros((128, 128), np.float32)
        normones[0] = 1.0
        normones[64] = 1.0
        in_maps.append(
            {
                "xT": xT,
                "wqkv": np.ascontiguousarray(W.T.astype(bf)),
                "projT": np.ascontiguousarray(pT.astype(bf)),
                "identb": identb,
                "normones": normones,
                "bias": np.ascontiguousarray(B),
            }
        )
    return in_maps


def kernel(x, wqkv_w, wqkv_b, proj_w, proj_b, _trace=False, _tmpdir=None):
    from concourse.bass_utils import run_bass_kernel_spmd

    nc = _get_nc()
    in_maps = _make_in_maps(x, wqkv_w, wqkv_b, proj_w)
    res = run_bass_kernel_spmd(
        nc,
        in_maps,
        core_ids=list(range(N_CORES)),
        trace=_trace,
        tmpdir=_tmpdir,
    )
    acc = np.zeros((NT, 8, 128, CH), np.float64)
    for rmap in res.results:
        acc += rmap["out"].astype(np.float64)
    partialT = acc.transpose(1, 2, 0, 3).reshape(C, T)  # [o, t]
    full = partialT.T + np.asarray(proj_b, np.float64)[None, :]
    if _trace:
        _CACHE["last_result"] = res
    return full.astype(np.float32)
